# revision 10
# baseline (speedup 1.0000x reference)
"""Trainium2 Bass kernel for nn_CrossAttention (B=2, N=2048, D=1024, H=16).

Sharding (8 cores): core c -> (batch b = c//4, head-group hg = c%4).
Each head-group is 4 heads = 256 of the 1024 projection dims.

Per core:
  QT = (Wq_hg @ X_b^T)      [256, 2048]   (transposed projections)
  KT = (Wk_hg @ X_b^T)      [256, 2048]
  V  = (X_b @ Wv_hg^T)      [2048, 256]   (natural orientation, + ones col)
  per head h, q-tile: ST = KT_h_tile^T-style scores [tok_k, tok_q] on PE,
  exp on ScalarE (scale folded, no max subtraction: logits ~ N(0,1)),
  PV matmul with ones-augmented V gives x^T and softmax denominators,
  out_partial = x_hg @ Wo_hg^T + bo/4   [2048, 1024]
  ReduceScatter(add) over the 4 cores of the batch -> each core owns a
  disjoint 512-token slice of the final output; host concatenates.

All matmuls run as float32r (FP22 reduced precision, 1 cycle/row on PE).
"""

import numpy as np

B = 2
NT = 2048
D = 1024
HEADS = 16
DH = 64
NCORES = 8
CPB = 4  # cores per batch
HG = HEADS // NCORES * 2  # 4 heads per core
HGD = HG * DH  # 256 cols per core
GROUPS = [[0, 1, 2, 3], [4, 5, 6, 7]]
SCALE = DH ** -0.5

_patched = False


def _patch_tile_drain():
    """This container's walrus rejects >1 sync-wait on a Drain
    (CoreV3GenImpl setupSyncWait<CTRL_NO_STRUCT>: "Too many sync wait
    commands").  Split the final TileContext drain's waits across a chain
    of single-wait drains; semaphores are monotonic so sequential waits
    are equivalent to one multi-wait."""
    global _patched
    if _patched:
        return
    import concourse.tile as tile
    import concourse.mybir as mybir
    from concourse.vector_clock import ScopedClock

    _uid = [0]

    def _split_multiwaits(nc):
        # Walrus here allows only ONE sync-wait per instruction; hoist
        # extra waits onto single-wait NoOps inserted just before, on the
        # same engine (engine execution is serial, sems are monotonic).
        for f in nc.m.functions:
            for bb in f.blocks:
                il = bb.instructions
                i = 0
                while i < len(il):
                    inst = il[i]
                    si = inst.sync_info
                    if si is not None and len(si.on_wait) > 1:
                        waits = list(si.on_wait)
                        inst.sync_info = mybir.SyncInfo(
                            on_wait=[waits[-1]], on_update=list(si.on_update)
                        )
                        for w in waits[:-1]:
                            _uid[0] += 1
                            nop = mybir.InstEventSemaphore(
                                name=f"WSPLIT-{_uid[0]}",
                                engine=inst.engine,
                                ins=[],
                                outs=[],
                                sync_info=mybir.SyncInfo(
                                    on_wait=[w], on_update=[]),
                            )
                            il.insert(i, nop)
                            i += 1
                    i += 1

    def _drain_and_barrier(self, tick_clock, wait_clock):
        nc = self.nc
        drain_inst = nc.sync.drain()
        wait_clock.add_sem_waits(
            drain_inst.ins, ScopedClock({None: tick_clock.global_clock})
        )
        si = drain_inst.ins.sync_info
        if si is not None and len(si.on_wait) > 1:
            waits = list(si.on_wait)
            drain_inst.ins.sync_info = mybir.SyncInfo(
                on_wait=[waits[0]], on_update=list(si.on_update)
            )
            for w in waits[1:]:
                extra = nc.sync.drain()
                extra.ins.sync_info = mybir.SyncInfo(on_wait=[w], on_update=[])

        _split_multiwaits(nc)
        nc.all_engine_barrier()
        assert self.sems is not None
        popped = nc._tile_sem_poison_stack.pop()
        assert popped is self._sem_poison
        nc.clear_and_free_semaphores(list(self.sems.allocated().values()))
        nc.all_engine_barrier()

    tile.TileContext._drain_and_barrier = _drain_and_barrier
    _patched = True


def build_program(nt=NT):
    """Build the SPMD Bass program (one NeuronCore's view)."""
    _patch_tile_drain()
    import concourse.bass as bass
    import concourse.tile as tile
    import concourse.mybir as mybir

    f32 = mybir.dt.float32
    f32r = mybir.dt.float32r
    EXP = mybir.ActivationFunctionType.Exp

    NQT = nt // 512   # q tiles (rhs free dim 512)
    NKT = nt // 128   # k tiles (PE contraction dim 128)
    NMT = nt // 128   # token m-tiles
    QSL = nt // CPB   # output token slice per core

    nc = bass.Bass("TRN2", target_bir_lowering=False, debug=False,
                   num_devices=NCORES)

    xqT = nc.dram_tensor("xqT", [D, nt], f32r, kind="ExternalInput")
    xkT = nc.dram_tensor("xkT", [D, nt], f32r, kind="ExternalInput")
    xvT = nc.dram_tensor("xvT", [D, nt], f32r, kind="ExternalInput")
    wqT = nc.dram_tensor("wqT", [D, HGD], f32r, kind="ExternalInput")
    wkT = nc.dram_tensor("wkT", [D, HGD], f32r, kind="ExternalInput")
    wvT = nc.dram_tensor("wvT", [D, HGD], f32r, kind="ExternalInput")
    woT = nc.dram_tensor("woT", [HGD, D], f32r, kind="ExternalInput")
    bo4 = nc.dram_tensor("bo4", [D], f32, kind="ExternalInput")
    out = nc.dram_tensor("out", [QSL, D], f32, kind="ExternalOutput")

    partial = nc.dram_tensor("partial", [nt, D], f32)
    rsout = nc.dram_tensor("rsout", [QSL, D], f32)
    rbounce = nc.dram_tensor("rbounce", [16, 512], f32)

    with tile.TileContext(nc) as tc:
        from contextlib import ExitStack
        with ExitStack() as ctx:
            const = ctx.enter_context(tc.tile_pool(name="const", bufs=1))
            persist = ctx.enter_context(tc.tile_pool(name="persist", bufs=1))
            rhs_pool = ctx.enter_context(tc.tile_pool(name="rhs", bufs=4))
            pt_pool = ctx.enter_context(tc.tile_pool(name="pt", bufs=4))
            misc = ctx.enter_context(tc.tile_pool(name="misc", bufs=4))
            outsb = ctx.enter_context(tc.tile_pool(name="outsb", bufs=3))
            # PSUM: 8 banks of [128, 512]f32 total.  One shared 4-slot pool
            # for all plain matmul accumulators ("mm" tag), 2 slots for the
            # attention x^T accumulators, 2 for the output projection.
            st_ps = ctx.enter_context(
                tc.tile_pool(name="st_ps", bufs=4, space="PSUM"))
            xa_ps = ctx.enter_context(
                tc.tile_pool(name="xa_ps", bufs=2, space="PSUM"))
            op_ps = ctx.enter_context(
                tc.tile_pool(name="op_ps", bufs=2, space="PSUM"))

            # --- constants -------------------------------------------------
            wq_sb = const.tile([128, 8, HGD], f32r)   # [k-part, k-tile, col]
            wk_sb = const.tile([128, 8, HGD], f32r)
            wv_sb = const.tile([128, 8, HGD], f32r)
            wo_sb = const.tile([128, 2, D], f32r)     # [d-part, hg k-tile, odim]
            bias_sb = const.tile([128, D], f32)      # bo/4 broadcast over parts
            nc.sync.dma_start(out=wq_sb[:], in_=wqT[:].rearrange(
                "(t p) c -> p t c", p=128))
            nc.sync.dma_start(out=wk_sb[:], in_=wkT[:].rearrange(
                "(t p) c -> p t c", p=128))
            nc.sync.dma_start(out=wv_sb[:], in_=wvT[:].rearrange(
                "(t p) c -> p t c", p=128))
            nc.sync.dma_start(out=wo_sb[:], in_=woT[:].rearrange(
                "(t p) c -> p t c", p=128))
            nc.sync.dma_start(out=bias_sb[:],
                              in_=bo4[:].partition_broadcast(128))
            ones_sb = const.tile([128, 1], f32)
            nc.vector.memset(ones_sb[:], 1.0)

            # --- persistent activations -----------------------------------
            qt_sb = persist.tile([128, 2, nt], f32r)   # QT: [qcol%128, qcol//128, tok]
            kt_sb = persist.tile([128, 2, nt], f32r)
            v_sb = persist.tile([128, NMT, HG * (DH + 1)], f32r)  # + ones col
            xt_sb = persist.tile([128, 2, nt], f32r)   # x^T (normalized)

            # --- Q/K projections: out[qcol, tok] = sum_k W^T[k,qcol] X^T[k,tok]
            for (src, wsb, dst) in ((xqT, wq_sb, qt_sb), (xkT, wk_sb, kt_sb)):
                for n in range(NQT):
                    acc = [st_ps.tile([128, 512], f32, tag="mm", name="qkacc")
                           for _ in range(2)]
                    for k in range(8):
                        rhs = rhs_pool.tile([128, 512], f32r, tag="projrhs")
                        nc.sync.dma_start(
                            out=rhs[:],
                            in_=src[128 * k:128 * (k + 1),
                                    512 * n:512 * (n + 1)])
                        for m in range(2):
                            nc.tensor.matmul(
                                acc[m][:],
                                wsb[:, k, 128 * m:128 * (m + 1)],
                                rhs[:],
                                start=(k == 0), stop=(k == 7))
                    for m in range(2):
                        nc.vector.tensor_copy(
                            dst[:, m, 512 * n:512 * (n + 1)], acc[m][:])

            # --- V projection: out[tok, vcol] = sum_k X^T[k,tok] W^T[k,vcol]
            for mg in range(nt // 512):
                acc = [st_ps.tile([128, 512], f32, tag="mm", name="vacc")
                       for _ in range(4)]
                for k in range(8):
                    lhs = rhs_pool.tile([128, 512], f32r, tag="projrhs")
                    nc.sync.dma_start(
                        out=lhs[:],
                        in_=xvT[128 * k:128 * (k + 1),
                                512 * mg:512 * (mg + 1)])
                    for m in range(4):
                        nc.tensor.matmul(
                            acc[m][:, 0:HGD],
                            lhs[:, 128 * m:128 * (m + 1)],
                            wv_sb[:, k, :],
                            start=(k == 0), stop=(k == 7))
                for m in range(4):
                    mt = 4 * mg + m
                    for h in range(HG):
                        nc.vector.tensor_copy(
                            v_sb[:, mt, (DH + 1) * h:(DH + 1) * h + DH],
                            acc[m][:, DH * h:DH * (h + 1)])
                        nc.vector.tensor_copy(
                            v_sb[:, mt, (DH + 1) * h + DH:(DH + 1) * (h + 1)],
                            ones_sb[:])

            # --- attention + output projection, per q-tile ----------------
            for qt in range(NQT):
                qsl = slice(512 * qt, 512 * (qt + 1))
                for hp in range(2):
                    xa = [xa_ps.tile([DH + 1, 512], f32, tag="xa", name="xa")
                          for _ in range(2)]
                    for kt in range(NKT):
                        st = [st_ps.tile([128, 512], f32, tag="mm", name="st")
                              for _ in range(2)]
                        for j in range(2):
                            p0 = 64 * j
                            nc.tensor.matmul(
                                st[j][:],
                                kt_sb[p0:p0 + 64, hp,
                                      128 * kt:128 * (kt + 1)],
                                qt_sb[p0:p0 + 64, hp, qsl],
                                tile_position=(p0, 0))
                        for j in range(2):
                            h = 2 * hp + j
                            pt = pt_pool.tile([128, 512], f32r)
                            nc.scalar.activation(pt[:], st[j][:], EXP,
                                                 scale=SCALE)
                            nc.tensor.matmul(
                                xa[j][:],
                                v_sb[:, kt,
                                     (DH + 1) * h:(DH + 1) * (h + 1)
                                     ],
                                pt[:],
                                start=(kt == 0), stop=(kt == NKT - 1))
                    # normalize: x[d, tok] /= sums[tok] (sums in row DH)
                    for j in range(2):
                        rr = misc.tile([DH + 1, 512], f32, tag="rr")
                        nc.vector.reciprocal(rr[DH:DH + 1, :],
                                             xa[j][DH:DH + 1, :])
                        bc = misc.tile([DH, 512], f32, tag="bc")
                        rb = rbounce[(4 * (qt % 2) + 2 * hp + j)
                                     % 16:(4 * (qt % 2) + 2 * hp + j) % 16 + 1,
                                     :]
                        nc.sync.dma_start(out=rb, in_=rr[DH:DH + 1, :])
                        nc.sync.dma_start(out=bc[:],
                                          in_=rb.partition_broadcast(DH))
                        if j == 0:
                            nc.vector.tensor_mul(
                                xt_sb[0:DH, hp, qsl], xa[j][0:DH, :], bc[:])
                        else:
                            tm = misc.tile([DH, 512], f32r, tag="tm")
                            nc.vector.tensor_mul(tm[:], xa[j][0:DH, :], bc[:])
                            nc.sync.dma_start(out=xt_sb[DH:128, hp, qsl],
                                              in_=tm[:])

                # out-proj for this q-tile:
                # partial[t, o] = sum_d x^T[d, t] Wo^T[d, o]  (+ bo/4)
                for n in range(2):
                    osl = slice(512 * n, 512 * (n + 1))
                    for m in range(4):
                        tsl = slice(512 * qt + 128 * m,
                                    512 * qt + 128 * (m + 1))
                        acc = op_ps.tile([128, 512], f32, tag="op")
                        for k in range(2):
                            nc.tensor.matmul(
                                acc[:],
                                xt_sb[:, k, tsl],
                                wo_sb[:, k, osl],
                                start=(k == 0), stop=(k == 1))
                        ob = outsb.tile([128, 512], f32)
                        nc.vector.tensor_add(ob[:], acc[:], bias_sb[:, osl])
                        nc.sync.dma_start(out=partial[tsl, osl], in_=ob[:])

            # --- reduce-scatter over the 4 cores of this batch ------------
            cc = nc.gpsimd.collective_compute(
                "ReduceScatter",
                mybir.AluOpType.add,
                replica_groups=GROUPS,
                ins=[partial[:]],
                outs=[rsout[:]],
            )
            fin = nc.sync.dma_start(out=out[:], in_=rsout[:])
            tile.add_dep_helper(fin.ins, cc.ins, reason="out after RS")

    return nc


_CACHE = {}


def _get_program(nt=NT):
    if nt not in _CACHE:
        _CACHE[nt] = build_program(nt)
    return _CACHE[nt]


def make_in_maps(query, key, value, Wq, Wk, Wv, Wo, bo, nt=NT):
    """Host-side sharding: per-core input dicts."""
    query = np.asarray(query, dtype=np.float32)
    key = np.asarray(key, dtype=np.float32)
    value = np.asarray(value, dtype=np.float32)
    Wq = np.asarray(Wq, dtype=np.float32)
    Wk = np.asarray(Wk, dtype=np.float32)
    Wv = np.asarray(Wv, dtype=np.float32)
    Wo = np.asarray(Wo, dtype=np.float32)
    bo = np.asarray(bo, dtype=np.float32)

    xT = [np.ascontiguousarray(x.T) for x in
          (query[0], key[0], value[0], query[1], key[1], value[1])]
    bo4 = (bo * 0.25).astype(np.float32)
    in_maps = []
    for c in range(NCORES):
        b, hg = divmod(c, CPB)
        hsl = slice(HGD * hg, HGD * (hg + 1))
        in_maps.append({
            "xqT": xT[3 * b + 0],
            "xkT": xT[3 * b + 1],
            "xvT": xT[3 * b + 2],
            "wqT": np.ascontiguousarray(Wq[hsl, :].T),
            "wkT": np.ascontiguousarray(Wk[hsl, :].T),
            "wvT": np.ascontiguousarray(Wv[hsl, :].T),
            "woT": np.ascontiguousarray(Wo[:, hsl].T),
            "bo4": bo4,
        })
    return in_maps


def assemble(results, nt=NT):
    """Concatenate per-core disjoint token slices into [B, NT, D]."""
    out = np.empty((B, nt, D), dtype=np.float32)
    qsl = nt // CPB
    for c in range(NCORES):
        b, p = divmod(c, CPB)
        out[b, qsl * p:qsl * (p + 1), :] = results[c]["out"]
    return out


def run(query, key, value, Wq, Wk, Wv, Wo, bo, nt=NT, trace=False):
    from concourse.bass_utils import run_bass_kernel_spmd
    nc = _get_program(nt)
    in_maps = make_in_maps(query, key, value, Wq, Wk, Wv, Wo, bo, nt=nt)
    res = run_bass_kernel_spmd(nc, in_maps, core_ids=list(range(NCORES)),
                               trace=trace)
    return assemble(res.results, nt=nt), res


def kernel(query, key, value, qpos=None, kpos=None, Wq=None, Wk=None,
           Wv=None, Wo=None, bo=None):
    out, _ = run(query, key, value, Wq, Wk, Wv, Wo, bo)
    return out
